# revision 14
# baseline (speedup 1.0000x reference)
"""Trainium2 Bass kernel: multi-head attention block (QKV proj + RoPE +
softmax attention + output proj).

Full shapes: hidden_states [4, 2048, 1024], Wq/Wk/Wv/Wo [1024, 1024],
16 heads x 64 dims. Sharding over 8 cores: data-parallel over batch (4)
x tensor-parallel over heads (2 groups of 8 heads). Each core computes a
partial output projection for its head group; the host sums the two
partials per batch.

Per-core layout strategy (everything "dim-major" = feature dim on SBUF
partitions, tokens on the free axis):
  Q^T, K^T  [512, 2048]  (8 local heads x 64 dims on partitions)
  V         [2048, 520]  token-major, 65 cols/head (col 64 = ones, so the
                          PV matmul also produces the softmax denominator)
  scores^T  = K^T-tile.T @ Q^T  -> PSUM [128 k, q], exp on ScalarE,
  attn^T    = Vpad-tile.T @ P^T -> PSUM [65, q]  (row 64 = sum of exps)
  y^T       = WoT-tile.T @ A^T  -> PSUM [128 o, t] partial output
"""

import numpy as np
import ml_dtypes

import concourse.bass as bass
import concourse.tile as tile
from concourse import bacc, mybir
from concourse.bass_utils import run_bass_kernel_spmd

BF16 = mybir.dt.bfloat16
F32 = mybir.dt.float32

B, S, H, NH, HD = 4, 2048, 1024, 16, 64
ROPE_BASE = 10000.0
N_CORES = 8
DLOC = H // 2          # 512 dims per core (8 heads)
NH_LOC = NH // 2       # 8 local heads
HT = H // 128          # 8 hidden k-tiles
DT = DLOC // 128       # 4 dim-tiles (head pairs)
TT = S // 128          # 16 token tiles
TCH = S // 512         # 4 token chunks of 512
VW = HD + 1            # 65: V columns per head incl. ones column


def _build_nc():
    nc = bacc.Bacc(None, target_bir_lowering=False)

    xt = nc.dram_tensor("xt", [H, S], BF16, kind="ExternalInput")
    wqt = nc.dram_tensor("wqt", [H, DLOC], BF16, kind="ExternalInput")
    wkt = nc.dram_tensor("wkt", [H, DLOC], BF16, kind="ExternalInput")
    wvt = nc.dram_tensor("wvt", [H, DLOC], BF16, kind="ExternalInput")
    wot = nc.dram_tensor("wot", [DLOC, H], BF16, kind="ExternalInput")
    cosd = nc.dram_tensor("cosd", [128, S], F32, kind="ExternalInput")
    sinrd = nc.dram_tensor("sinrd", [128, S], F32, kind="ExternalInput")
    yt = nc.dram_tensor("yt", [H, S], F32, kind="ExternalOutput")

    EXP = mybir.ActivationFunctionType.Exp
    SCALE = float(1.0 / np.sqrt(HD))

    with tile.TileContext(nc) as tc:
        with tc.tile_pool(name="persist", bufs=1) as pp:
            # ---- load inputs (xt + wqt + cos/sin first: critical path) ------
            xts = []
            for i in range(HT):
                t = pp.tile([128, S], BF16, tag=f"xt{i}", name=f"xt{i}")
                nc.sync.dma_start(t[:], xt[i * 128:(i + 1) * 128, :])
                xts.append(t)
            wqts, wkts, wvts = [], [], []
            for i in range(HT):
                t = pp.tile([128, DLOC], BF16, tag=f"wq{i}", name=f"wq{i}")
                nc.sync.dma_start(t[:], wqt[i * 128:(i + 1) * 128, :])
                wqts.append(t)
            cos_t = pp.tile([128, S], F32, tag="cos")
            nc.sync.dma_start(cos_t[:], cosd[:])
            sinr_t = pp.tile([128, S], F32, tag="sinr")
            nc.sync.dma_start(sinr_t[:], sinrd[:])
            for name, dram, lst in (("wk", wkt, wkts), ("wv", wvt, wvts)):
                for i in range(HT):
                    t = pp.tile([128, DLOC], BF16, tag=f"{name}{i}", name=f"{name}{i}")
                    nc.sync.dma_start(t[:], dram[i * 128:(i + 1) * 128, :])
                    lst.append(t)
            wots = []
            for i in range(DT):
                t = pp.tile([128, H], BF16, tag=f"wo{i}", name=f"wo{i}")
                nc.sync.dma_start(t[:], wot[i * 128:(i + 1) * 128, :])
                wots.append(t)

            qts = [pp.tile([128, S], BF16, tag=f"qt{i}", name=f"qt{i}") for i in range(DT)]
            kts = [pp.tile([128, S], BF16, tag=f"kt{i}", name=f"kt{i}") for i in range(DT)]
            vps = [pp.tile([128, NH_LOC * VW], BF16, tag=f"v{i}", name=f"v{i}")
                   for i in range(TT)]
            ats = [pp.tile([128, S], BF16, tag=f"at{i}", name=f"at{i}") for i in range(DT)]

            with tc.tile_pool(name="ps", bufs=2, space="PSUM") as psp, \
                 tc.tile_pool(name="pv_ps", bufs=2, space="PSUM") as pvps, \
                 tc.tile_pool(name="pt", bufs=6) as ptp, \
                 tc.tile_pool(name="rope_tmp", bufs=2) as rtp, \
                 tc.tile_pool(name="rc", bufs=2) as rcp:

                # warm the ACT exp table before it is on the critical path
                warm = rcp.tile([1, 8], F32, tag="warm")
                nc.vector.memset(warm[:], 0.0)
                nc.scalar.activation(warm[:], warm[:], EXP)

                def emit_v_proj():
                    for i in range(TT // 2):
                        ps = psp.tile([128, 1024], F32, tag="ps", name=f"psv{i}")
                        for j in range(2):
                            tt_i = 2 * i + j
                            for ht in range(HT):
                                nc.tensor.matmul(
                                    ps[:, j * 512:(j + 1) * 512],
                                    lhsT=xts[ht][:, tt_i * 128:(tt_i + 1) * 128],
                                    rhs=wvts[ht][:],
                                    start=(ht == 0), stop=(ht == HT - 1))
                        for j in range(2):
                            tt_i = 2 * i + j
                            v3 = vps[tt_i][:].rearrange("p (h d) -> p h d", d=VW)
                            p3 = ps[:, j * 512:(j + 1) * 512].rearrange(
                                "p (h d) -> p h d", d=HD)
                            nc.vector.tensor_copy(v3[:, :, 0:HD], p3)
                            nc.vector.memset(v3[:, :, HD:VW], 1.0)

                def emit_qk_proj(pair):
                    """Generator: Q/K projection + RoPE for dim-tile `pair`,
                    yielding after each half (one PSUM-tile group)."""
                    for wts, outts, nm in ((wqts, qts, "q"), (wkts, kts, "k")):
                        for half in range(2):
                            tmp2 = rtp.tile([128, 1024], F32, tag="tmp2",
                                            name=f"tmp2{nm}{pair}{half}")
                            tmpc = rtp.tile([128, 1024], F32, tag="tmpc",
                                            name=f"tmpc{nm}{pair}{half}")
                            tmp3 = rtp.tile([128, 1024], F32, tag="tmp3",
                                            name=f"tmp3{nm}{pair}{half}")
                            ps = psp.tile([128, 1024], F32, tag="ps",
                                          name=f"ps{nm}{pair}{half}")
                            for j in range(2):
                                tch = half * 2 + j
                                for ht in range(HT):
                                    nc.tensor.matmul(
                                        ps[:, j * 512:(j + 1) * 512],
                                        lhsT=wts[ht][:, pair * 128:(pair + 1) * 128],
                                        rhs=xts[ht][:, tch * 512:(tch + 1) * 512],
                                        start=(ht == 0), stop=(ht == HT - 1))
                            hs_ = slice(half * 1024, (half + 1) * 1024)
                            for j in range(2):
                                tch = half * 2 + j
                                cs = slice(tch * 512, (tch + 1) * 512)
                                js = slice(j * 512, (j + 1) * 512)
                                nc.vector.tensor_mul(tmp2[:, js], ps[:, js],
                                                     sinr_t[:, cs])
                                nc.vector.tensor_mul(tmpc[:, js], ps[:, js],
                                                     cos_t[:, cs])
                            for blk in (0, 64):
                                nc.sync.dma_start(tmp3[blk:blk + 32, :],
                                                  tmp2[blk + 32:blk + 64, :])
                                nc.sync.dma_start(tmp3[blk + 32:blk + 64, :],
                                                  tmp2[blk:blk + 32, :])
                            nc.vector.tensor_add(outts[pair][:, hs_], tmpc[:],
                                                 tmp3[:])
                            yield

                LAG = 3  # PV trails exp by LAG k-tiles so PE never waits on ACT

                def emit_attention(h, qc):
                    p, r = h // 2, h % 2
                    rb = r * 64
                    q0 = qc * 1024
                    pv = pvps.tile([VW, 1024], F32, tag="pv", name=f"pv{h}_{qc}")
                    pts = {}
                    for kt_i in range(TT + LAG):
                        if kt_i < TT:
                            qk = psp.tile([128, 1024], F32, tag="ps",
                                          name=f"qk{h}_{qc}_{kt_i}")
                            for j in range(2):
                                nc.tensor.matmul(
                                    qk[:, j * 512:(j + 1) * 512],
                                    lhsT=kts[p][rb:rb + 64,
                                                kt_i * 128:(kt_i + 1) * 128],
                                    rhs=qts[p][rb:rb + 64,
                                               q0 + j * 512:q0 + (j + 1) * 512],
                                    start=True, stop=True)
                            pt = ptp.tile([128, 1024], BF16, tag="pt",
                                          name=f"pt{h}_{qc}_{kt_i}")
                            nc.scalar.activation(pt[:], qk[:], EXP, scale=SCALE)
                            pts[kt_i] = pt
                        if kt_i >= LAG:
                            kv = kt_i - LAG
                            for j in range(2):
                                nc.tensor.matmul(
                                    pv[:, j * 512:(j + 1) * 512],
                                    lhsT=vps[kv][:, h * VW:(h + 1) * VW],
                                    rhs=pts[kv][:, j * 512:(j + 1) * 512],
                                    start=(kv == 0), stop=(kv == TT - 1))
                            del pts[kv]
                    p_, r_ = h // 2, h % 2
                    sums = rcp.tile([1, 1024], F32, tag="sums", name=f"sm{h}{qc}")
                    nc.vector.tensor_copy(sums[:], pv[HD:VW, :])
                    recip = rcp.tile([1, 1024], F32, tag="recip", name=f"rp{h}{qc}")
                    nc.vector.reciprocal_approx_fast(recip[:], sums[:])
                    recip64 = rcp.tile([64, 1024], F32, tag="recip64",
                                       name=f"rp64{h}{qc}")
                    nc.gpsimd.partition_broadcast(recip64[:], recip[:], channels=64)
                    nc.vector.tensor_mul(ats[p_][rb:rb + 64, q0:q0 + 1024],
                                         pv[0:HD, :], recip64[:])

                # ---- emission schedule -------------------------------------
                INTERLEAVE = True
                emit_v_proj()
                for _ in emit_qk_proj(0):
                    pass
                for pair in range(DT):
                    nxt = iter(emit_qk_proj(pair + 1)) if pair + 1 < DT else None
                    if not INTERLEAVE and nxt is not None:
                        for _ in nxt:
                            pass
                        nxt = None
                    for h in (2 * pair, 2 * pair + 1):
                        for qc in range(2):
                            emit_attention(h, qc)
                            if nxt is not None:
                                next(nxt, None)
                    if nxt is not None:
                        for _ in nxt:
                            pass

            # ---- output projection ------------------------------------------
            with tc.tile_pool(name="o_ps", bufs=4, space="PSUM") as ops, \
                 tc.tile_pool(name="y", bufs=4) as yp:
                for ot in range(HT):
                    for tch in range(TCH):
                        ps = ops.tile([128, 512], F32, tag="ps", name=f"yps{ot}{tch}")
                        for dt_i in range(DT):
                            nc.tensor.matmul(
                                ps[:],
                                lhsT=wots[dt_i][:, ot * 128:(ot + 1) * 128],
                                rhs=ats[dt_i][:, tch * 512:(tch + 1) * 512],
                                start=(dt_i == 0), stop=(dt_i == DT - 1))
                        ysb = yp.tile([128, 512], F32, tag="y", name=f"y{ot}{tch}")
                        nc.vector.tensor_copy(ysb[:], ps[:])
                        nc.sync.dma_start(
                            yt[ot * 128:(ot + 1) * 128,
                               tch * 512:(tch + 1) * 512], ysb[:])

    nc.compile()
    return nc


_NC = None


def _get_nc():
    global _NC
    if _NC is None:
        _NC = _build_nc()
    return _NC


def _host_inputs(hidden_states, Wq, Wk, Wv, Wo):
    bf = ml_dtypes.bfloat16
    inv = 1.0 / (ROPE_BASE ** (np.arange(0, HD, 2, dtype=np.float64) / HD))
    t = np.arange(S, dtype=np.float64)
    ang = np.outer(inv, t)                      # [32, S]
    cos32 = np.cos(ang).astype(np.float32)
    sin32 = np.sin(ang).astype(np.float32)
    cosd = np.tile(cos32, (4, 1))               # [128, S]
    # sinrot: +sin on lower half of each 64-block, -sin on upper half
    sinrd = np.tile(np.concatenate([sin32, -sin32], axis=0), (2, 1))
    cosd = np.ascontiguousarray(cosd, dtype=np.float32)
    sinrd = np.ascontiguousarray(sinrd, dtype=np.float32)

    WqT = np.ascontiguousarray(Wq.T).astype(bf)     # [H, H]
    WkT = np.ascontiguousarray(Wk.T).astype(bf)
    WvT = np.ascontiguousarray(Wv.T).astype(bf)
    WoT = np.ascontiguousarray(Wo.T).astype(bf)     # [H(d), H(o)]

    in_maps = []
    for c in range(N_CORES):
        b, g = c // 2, c % 2
        gs = slice(g * DLOC, (g + 1) * DLOC)
        in_maps.append({
            "xt": np.ascontiguousarray(hidden_states[b].T).astype(bf),
            "wqt": np.ascontiguousarray(WqT[:, gs]),
            "wkt": np.ascontiguousarray(WkT[:, gs]),
            "wvt": np.ascontiguousarray(WvT[:, gs]),
            "wot": np.ascontiguousarray(WoT[gs, :]),
            "cosd": cosd,
            "sinrd": sinrd,
        })
    return in_maps


def kernel(hidden_states, Wq, Wk, Wv, Wo, _trace=False, _tmpdir=None):
    nc = _get_nc()
    in_maps = _host_inputs(hidden_states, Wq, Wk, Wv, Wo)
    res = run_bass_kernel_spmd(nc, in_maps, core_ids=list(range(N_CORES)),
                               trace=_trace, tmpdir=_tmpdir)
    kernel._last_results = res
    out = np.empty((B, S, H), dtype=np.float32)
    for b in range(B):
        acc = res.results[2 * b]["yt"].astype(np.float32) \
            + res.results[2 * b + 1]["yt"].astype(np.float32)
        out[b] = acc.T
    return out


# revision 17
# speedup vs baseline: 1.3614x; 1.3614x over previous
"""Trainium2 Bass kernel: multi-head attention block (QKV proj + RoPE +
softmax attention + output proj).

Full shapes: hidden_states [4, 2048, 1024], Wq/Wk/Wv/Wo [1024, 1024],
16 heads x 64 dims. Sharding over 8 cores: data-parallel over batch (4)
x tensor-parallel over heads (2 groups of 8 heads). Each core computes a
partial output projection for its head group; the host sums the two
partials per batch.

Per-core layout strategy (everything "dim-major" = feature dim on SBUF
partitions, tokens on the free axis):
  Q^T, K^T  [512, 2048]  (8 local heads x 64 dims on partitions)
  V         [2048, 520]  token-major, 65 cols/head (col 64 = ones, so the
                          PV matmul also produces the softmax denominator)
  scores^T  = K^T-tile.T @ Q^T  -> PSUM [128 k, q], exp on ScalarE,
  attn^T    = Vpad-tile.T @ P^T -> PSUM [65, q]  (row 64 = sum of exps)
  y^T       = WoT-tile.T @ A^T  -> PSUM [128 o, t] partial output
"""

import numpy as np
import ml_dtypes

import concourse.bass as bass
import concourse.tile as tile
from concourse import bacc, mybir
from concourse.bass_utils import run_bass_kernel_spmd

BF16 = mybir.dt.bfloat16
F32 = mybir.dt.float32

B, S, H, NH, HD = 4, 2048, 1024, 16, 64
ROPE_BASE = 10000.0
N_CORES = 8
DLOC = H // 2          # 512 dims per core (8 heads)
NH_LOC = NH // 2       # 8 local heads
HT = H // 128          # 8 hidden k-tiles
DT = DLOC // 128       # 4 dim-tiles (head pairs)
TT = S // 128          # 16 token tiles
TCH = S // 512         # 4 token chunks of 512
VW = HD + 1            # 65: V columns per head incl. ones column


def _build_nc():
    nc = bacc.Bacc(None, target_bir_lowering=False)

    xt = nc.dram_tensor("xt", [H, S], BF16, kind="ExternalInput")
    wqt = nc.dram_tensor("wqt", [H, DLOC], BF16, kind="ExternalInput")
    wkt = nc.dram_tensor("wkt", [H, DLOC], BF16, kind="ExternalInput")
    wvt = nc.dram_tensor("wvt", [H, DLOC], BF16, kind="ExternalInput")
    wot = nc.dram_tensor("wot", [DLOC, H], BF16, kind="ExternalInput")
    cosd = nc.dram_tensor("cosd", [128, S], F32, kind="ExternalInput")
    sinrd = nc.dram_tensor("sinrd", [128, S], F32, kind="ExternalInput")
    yt = nc.dram_tensor("yt", [H, S], F32, kind="ExternalOutput")

    EXP = mybir.ActivationFunctionType.Exp
    SCALE = float(1.0 / np.sqrt(HD))

    with tile.TileContext(nc) as tc:
        with tc.tile_pool(name="persist", bufs=1) as pp:
            # ---- load inputs (xt + wqt + cos/sin first: critical path) ------
            xts = []
            for i in range(HT):
                t = pp.tile([128, S], BF16, tag=f"xt{i}", name=f"xt{i}")
                nc.sync.dma_start(t[:], xt[i * 128:(i + 1) * 128, :])
                xts.append(t)
            wqts, wkts, wvts = [], [], []
            for i in range(HT):
                t = pp.tile([128, DLOC], BF16, tag=f"wq{i}", name=f"wq{i}")
                nc.sync.dma_start(t[:], wqt[i * 128:(i + 1) * 128, :])
                wqts.append(t)
            cos_t = pp.tile([128, S], F32, tag="cos")
            nc.sync.dma_start(cos_t[:], cosd[:])
            sinr_t = pp.tile([128, S], F32, tag="sinr")
            nc.sync.dma_start(sinr_t[:], sinrd[:])
            for name, dram, lst in (("wk", wkt, wkts), ("wv", wvt, wvts)):
                for i in range(HT):
                    t = pp.tile([128, DLOC], BF16, tag=f"{name}{i}", name=f"{name}{i}")
                    nc.sync.dma_start(t[:], dram[i * 128:(i + 1) * 128, :])
                    lst.append(t)
            wots = []
            for i in range(DT):
                t = pp.tile([128, H], BF16, tag=f"wo{i}", name=f"wo{i}")
                nc.sync.dma_start(t[:], wot[i * 128:(i + 1) * 128, :])
                wots.append(t)

            qts = [pp.tile([128, S], BF16, tag=f"qt{i}", name=f"qt{i}") for i in range(DT)]
            kts = [pp.tile([128, S], BF16, tag=f"kt{i}", name=f"kt{i}") for i in range(DT)]
            vps = [pp.tile([128, NH_LOC * VW], BF16, tag=f"v{i}", name=f"v{i}")
                   for i in range(TT)]
            ats = [pp.tile([128, S], BF16, tag=f"at{i}", name=f"at{i}") for i in range(DT)]

            with tc.tile_pool(name="proj_ps", bufs=2, space="PSUM") as prp, \
                 tc.tile_pool(name="qk_ps", bufs=2, space="PSUM") as qkps, \
                 tc.tile_pool(name="pv_ps", bufs=1, space="PSUM") as pvps, \
                 tc.tile_pool(name="pt", bufs=6) as ptp, \
                 tc.tile_pool(name="rope_tmp", bufs=2) as rtp, \
                 tc.tile_pool(name="au", bufs=2) as aup, \
                 tc.tile_pool(name="rc", bufs=2) as rcp:

                # warm the ACT exp table before it is on the critical path
                warm = rcp.tile([1, 8], F32, tag="warm")
                nc.vector.memset(warm[:], 0.0)
                nc.scalar.activation(warm[:], warm[:], EXP)

                def emit_v_proj():
                    for tt_i in range(TT):
                        ps = prp.tile([128, 512], F32, tag="ps", name=f"psv{tt_i}")
                        for ht in range(HT):
                            nc.tensor.matmul(
                                ps[:],
                                lhsT=xts[ht][:, tt_i * 128:(tt_i + 1) * 128],
                                rhs=wvts[ht][:],
                                start=(ht == 0), stop=(ht == HT - 1))
                        v3 = vps[tt_i][:].rearrange("p (h d) -> p h d", d=VW)
                        p3 = ps[:].rearrange("p (h d) -> p h d", d=HD)
                        nc.vector.tensor_copy(v3[:, :, 0:HD], p3)
                        nc.vector.memset(v3[:, :, HD:VW], 1.0)

                def emit_qk_proj(pair):
                    """Generator: Q/K projection + RoPE for dim-tile `pair`.
                    Yields after each 8-matmul PSUM group (8 yields total)."""
                    for wts, outts, nm in ((wqts, qts, "q"), (wkts, kts, "k")):
                        for half in range(2):
                            tmp2 = rtp.tile([128, 1024], F32, tag="tmp2",
                                            name=f"tmp2{nm}{pair}{half}")
                            tmpc = rtp.tile([128, 1024], F32, tag="tmpc",
                                            name=f"tmpc{nm}{pair}{half}")
                            tmp3 = rtp.tile([128, 1024], F32, tag="tmp3",
                                            name=f"tmp3{nm}{pair}{half}")
                            for j in range(2):
                                tch = half * 2 + j
                                ps = prp.tile([128, 512], F32, tag="ps",
                                              name=f"ps{nm}{pair}{tch}")
                                for ht in range(HT):
                                    nc.tensor.matmul(
                                        ps[:],
                                        lhsT=wts[ht][:, pair * 128:(pair + 1) * 128],
                                        rhs=xts[ht][:, tch * 512:(tch + 1) * 512],
                                        start=(ht == 0), stop=(ht == HT - 1))
                                cs = slice(tch * 512, (tch + 1) * 512)
                                js = slice(j * 512, (j + 1) * 512)
                                nc.vector.tensor_mul(tmp2[:, js], ps[:], sinr_t[:, cs])
                                nc.vector.tensor_mul(tmpc[:, js], ps[:], cos_t[:, cs])
                                yield
                            hs_ = slice(half * 1024, (half + 1) * 1024)
                            for blk in (0, 64):
                                nc.sync.dma_start(tmp3[blk:blk + 32, :],
                                                  tmp2[blk + 32:blk + 64, :])
                                nc.sync.dma_start(tmp3[blk + 32:blk + 64, :],
                                                  tmp2[blk:blk + 32, :])
                            nc.vector.tensor_add(outts[pair][:, hs_], tmpc[:],
                                                 tmp3[:])

                LAG = 3  # PV trails exp by LAG k-tiles so PE never waits on ACT

                def emit_attention(h, qc):
                    p, r = h // 2, h % 2
                    rb = r * 64
                    q0 = qc * 1024
                    pv = pvps.tile([VW, 1024], F32, tag="pv", name=f"pv{h}_{qc}")
                    pts = {}
                    for kt_i in range(TT + LAG):
                        if kt_i < TT:
                            qk = qkps.tile([128, 1024], F32, tag="qk",
                                           name=f"qk{h}_{qc}_{kt_i}")
                            for j in range(2):
                                nc.tensor.matmul(
                                    qk[:, j * 512:(j + 1) * 512],
                                    lhsT=kts[p][rb:rb + 64,
                                                kt_i * 128:(kt_i + 1) * 128],
                                    rhs=qts[p][rb:rb + 64,
                                               q0 + j * 512:q0 + (j + 1) * 512],
                                    start=True, stop=True)
                            pt = ptp.tile([128, 1024], BF16, tag="pt",
                                          name=f"pt{h}_{qc}_{kt_i}")
                            nc.scalar.activation(pt[:], qk[:], EXP, scale=SCALE)
                            pts[kt_i] = pt
                        if kt_i >= LAG:
                            kv = kt_i - LAG
                            for j in range(2):
                                nc.tensor.matmul(
                                    pv[:, j * 512:(j + 1) * 512],
                                    lhsT=vps[kv][:, h * VW:(h + 1) * VW],
                                    rhs=pts[kv][:, j * 512:(j + 1) * 512],
                                    start=(kv == 0), stop=(kv == TT - 1))
                            del pts[kv]
                    # copy unnormalized A^T + sums to SBUF: frees the PV bank
                    # immediately; normalization happens off the critical path
                    au = aup.tile([HD, 1024], F32, tag="au", name=f"au{h}_{qc}")
                    nc.vector.tensor_copy(au[:], pv[0:HD, :])
                    sums = rcp.tile([1, 1024], F32, tag="sums", name=f"sm{h}{qc}", bufs=1)
                    nc.vector.tensor_copy(sums[:], pv[HD:VW, :])
                    recip = rcp.tile([1, 1024], F32, tag="recip", name=f"rp{h}{qc}", bufs=1)
                    nc.vector.reciprocal_approx_fast(recip[:], sums[:])
                    recip64 = rcp.tile([64, 1024], F32, tag="recip64",
                                       name=f"rp64{h}{qc}", bufs=1)
                    nc.gpsimd.partition_broadcast(recip64[:], recip[:], channels=64)
                    nc.vector.tensor_mul(ats[p][rb:rb + 64, q0:q0 + 1024],
                                         au[:], recip64[:])

                # ---- emission schedule -------------------------------------
                emit_v_proj()
                for _ in emit_qk_proj(0):
                    pass
                for pair in range(DT):
                    nxt = iter(emit_qk_proj(pair + 1)) if pair + 1 < DT else None
                    for h in (2 * pair, 2 * pair + 1):
                        for qc in range(2):
                            emit_attention(h, qc)
                            if nxt is not None:
                                next(nxt, None)
                                next(nxt, None)
                    if nxt is not None:
                        for _ in nxt:
                            pass

            # ---- output projection ------------------------------------------
            with tc.tile_pool(name="o_ps", bufs=4, space="PSUM") as ops, \
                 tc.tile_pool(name="y", bufs=4) as yp:
                for ot in range(HT):
                    for tch in range(TCH):
                        ps = ops.tile([128, 512], F32, tag="ps", name=f"yps{ot}{tch}")
                        for dt_i in range(DT):
                            nc.tensor.matmul(
                                ps[:],
                                lhsT=wots[dt_i][:, ot * 128:(ot + 1) * 128],
                                rhs=ats[dt_i][:, tch * 512:(tch + 1) * 512],
                                start=(dt_i == 0), stop=(dt_i == DT - 1))
                        ysb = yp.tile([128, 512], F32, tag="y", name=f"y{ot}{tch}")
                        nc.vector.tensor_copy(ysb[:], ps[:])
                        nc.sync.dma_start(
                            yt[ot * 128:(ot + 1) * 128,
                               tch * 512:(tch + 1) * 512], ysb[:])

    nc.compile()
    return nc


_NC = None


def _get_nc():
    global _NC
    if _NC is None:
        _NC = _build_nc()
    return _NC


def _host_inputs(hidden_states, Wq, Wk, Wv, Wo):
    bf = ml_dtypes.bfloat16
    inv = 1.0 / (ROPE_BASE ** (np.arange(0, HD, 2, dtype=np.float64) / HD))
    t = np.arange(S, dtype=np.float64)
    ang = np.outer(inv, t)                      # [32, S]
    cos32 = np.cos(ang).astype(np.float32)
    sin32 = np.sin(ang).astype(np.float32)
    cosd = np.tile(cos32, (4, 1))               # [128, S]
    # sinrot: +sin on lower half of each 64-block, -sin on upper half
    sinrd = np.tile(np.concatenate([sin32, -sin32], axis=0), (2, 1))
    cosd = np.ascontiguousarray(cosd, dtype=np.float32)
    sinrd = np.ascontiguousarray(sinrd, dtype=np.float32)

    WqT = np.ascontiguousarray(Wq.T).astype(bf)     # [H, H]
    WkT = np.ascontiguousarray(Wk.T).astype(bf)
    WvT = np.ascontiguousarray(Wv.T).astype(bf)
    WoT = np.ascontiguousarray(Wo.T).astype(bf)     # [H(d), H(o)]

    in_maps = []
    for c in range(N_CORES):
        b, g = c // 2, c % 2
        gs = slice(g * DLOC, (g + 1) * DLOC)
        in_maps.append({
            "xt": np.ascontiguousarray(hidden_states[b].T).astype(bf),
            "wqt": np.ascontiguousarray(WqT[:, gs]),
            "wkt": np.ascontiguousarray(WkT[:, gs]),
            "wvt": np.ascontiguousarray(WvT[:, gs]),
            "wot": np.ascontiguousarray(WoT[gs, :]),
            "cosd": cosd,
            "sinrd": sinrd,
        })
    return in_maps


def kernel(hidden_states, Wq, Wk, Wv, Wo, _trace=False, _tmpdir=None):
    nc = _get_nc()
    in_maps = _host_inputs(hidden_states, Wq, Wk, Wv, Wo)
    res = run_bass_kernel_spmd(nc, in_maps, core_ids=list(range(N_CORES)),
                               trace=_trace, tmpdir=_tmpdir)
    kernel._last_results = res
    out = np.empty((B, S, H), dtype=np.float32)
    for b in range(B):
        acc = res.results[2 * b]["yt"].astype(np.float32) \
            + res.results[2 * b + 1]["yt"].astype(np.float32)
        out[b] = acc.T
    return out


# revision 19
# speedup vs baseline: 1.4086x; 1.0346x over previous
"""Trainium2 Bass kernel: multi-head attention block (QKV proj + RoPE +
softmax attention + output proj).

Full shapes: hidden_states [4, 2048, 1024], Wq/Wk/Wv/Wo [1024, 1024],
16 heads x 64 dims. Sharding over 8 cores: data-parallel over batch (4)
x tensor-parallel over heads (2 groups of 8 heads). Each core computes a
partial output projection for its head group; the host sums the two
partials per batch.

Per-core layout strategy (everything "dim-major" = feature dim on SBUF
partitions, tokens on the free axis):
  Q^T, K^T  [512, 2048]  (8 local heads x 64 dims on partitions)
  V         [2048, 520]  token-major, 65 cols/head (col 64 = ones, so the
                          PV matmul also produces the softmax denominator)
  scores^T  = K^T-tile.T @ Q^T  -> PSUM [128 k, q], exp on ScalarE,
  attn^T    = Vpad-tile.T @ P^T -> PSUM [65, q]  (row 64 = sum of exps)
  y^T       = WoT-tile.T @ A^T  -> PSUM [128 o, t] partial output
"""

import numpy as np
import ml_dtypes

import concourse.bass as bass
import concourse.tile as tile
from concourse import bacc, mybir
from concourse.bass_utils import run_bass_kernel_spmd

BF16 = mybir.dt.bfloat16
F32 = mybir.dt.float32

B, S, H, NH, HD = 4, 2048, 1024, 16, 64
ROPE_BASE = 10000.0
N_CORES = 8
DLOC = H // 2          # 512 dims per core (8 heads)
NH_LOC = NH // 2       # 8 local heads
HT = H // 128          # 8 hidden k-tiles
DT = DLOC // 128       # 4 dim-tiles (head pairs)
TT = S // 128          # 16 token tiles
TCH = S // 512         # 4 token chunks of 512
VW = HD + 1            # 65: V columns per head incl. ones column


def _build_nc():
    nc = bacc.Bacc(None, target_bir_lowering=False)

    xt = nc.dram_tensor("xt", [H, S], BF16, kind="ExternalInput")
    wqt = nc.dram_tensor("wqt", [H, DLOC], BF16, kind="ExternalInput")
    wkt = nc.dram_tensor("wkt", [H, DLOC], BF16, kind="ExternalInput")
    wvt = nc.dram_tensor("wvt", [H, DLOC], BF16, kind="ExternalInput")
    wot = nc.dram_tensor("wot", [DLOC, H], BF16, kind="ExternalInput")
    cosd = nc.dram_tensor("cosd", [128, S], F32, kind="ExternalInput")
    sinrd = nc.dram_tensor("sinrd", [128, S], F32, kind="ExternalInput")
    yt = nc.dram_tensor("yt", [H, S], F32, kind="ExternalOutput")

    EXP = mybir.ActivationFunctionType.Exp
    SCALE = float(1.0 / np.sqrt(HD))

    with tile.TileContext(nc) as tc:
        with tc.tile_pool(name="persist", bufs=1) as pp:
            # ---- load inputs (xt + wqt + cos/sin first: critical path) ------
            xts, wqts, wkts, wvts, wots = [], [], [], [], []
            for i in range(HT):
                t = pp.tile([128, S], BF16, tag=f"xt{i}", name=f"xt{i}")
                nc.sync.dma_start(t[:], xt[i * 128:(i + 1) * 128, :])
                xts.append(t)
            for i in range(HT):
                t = pp.tile([128, DLOC], BF16, tag=f"wv{i}", name=f"wv{i}")
                nc.gpsimd.dma_start(t[:], wvt[i * 128:(i + 1) * 128, :])
                wvts.append(t)
            cos_t = pp.tile([128, S], F32, tag="cos")
            nc.gpsimd.dma_start(cos_t[:], cosd[:])
            sinr_t = pp.tile([128, S], F32, tag="sinr")
            nc.gpsimd.dma_start(sinr_t[:], sinrd[:])
            for i in range(HT):
                t = pp.tile([128, DLOC], BF16, tag=f"wq{i}", name=f"wq{i}")
                nc.sync.dma_start(t[:], wqt[i * 128:(i + 1) * 128, :])
                wqts.append(t)
            for i in range(HT):
                t = pp.tile([128, DLOC], BF16, tag=f"wk{i}", name=f"wk{i}")
                nc.sync.dma_start(t[:], wkt[i * 128:(i + 1) * 128, :])
                wkts.append(t)
            for i in range(DT):
                t = pp.tile([128, H], BF16, tag=f"wo{i}", name=f"wo{i}")
                nc.gpsimd.dma_start(t[:], wot[i * 128:(i + 1) * 128, :])
                wots.append(t)

            qts = [pp.tile([128, S], BF16, tag=f"qt{i}", name=f"qt{i}") for i in range(DT)]
            kts = [pp.tile([128, S], BF16, tag=f"kt{i}", name=f"kt{i}") for i in range(DT)]
            vps = [pp.tile([128, NH_LOC * VW], BF16, tag=f"v{i}", name=f"v{i}")
                   for i in range(TT)]
            ats = [pp.tile([128, S], BF16, tag=f"at{i}", name=f"at{i}") for i in range(DT)]

            with tc.tile_pool(name="proj_ps", bufs=2, space="PSUM") as prp, \
                 tc.tile_pool(name="qk_ps", bufs=2, space="PSUM") as qkps, \
                 tc.tile_pool(name="pv_ps", bufs=1, space="PSUM") as pvps, \
                 tc.tile_pool(name="pt", bufs=6) as ptp, \
                 tc.tile_pool(name="rope_tmp", bufs=2) as rtp, \
                 tc.tile_pool(name="au", bufs=2) as aup, \
                 tc.tile_pool(name="rc", bufs=2) as rcp:

                # warm the ACT exp table before it is on the critical path
                warm = rcp.tile([1, 8], F32, tag="warm")
                nc.vector.memset(warm[:], 0.0)
                nc.scalar.activation(warm[:], warm[:], EXP)

                def emit_v_proj():
                    for tt_i in range(TT):
                        ps = prp.tile([128, 512], F32, tag="ps", name=f"psv{tt_i}")
                        for ht in range(HT):
                            nc.tensor.matmul(
                                ps[:],
                                lhsT=xts[ht][:, tt_i * 128:(tt_i + 1) * 128],
                                rhs=wvts[ht][:],
                                start=(ht == 0), stop=(ht == HT - 1))
                        v3 = vps[tt_i][:].rearrange("p (h d) -> p h d", d=VW)
                        p3 = ps[:].rearrange("p (h d) -> p h d", d=HD)
                        nc.vector.tensor_copy(v3[:, :, 0:HD], p3)
                        nc.vector.memset(v3[:, :, HD:VW], 1.0)
                        yield

                def emit_qk_proj(pair):
                    """Generator: Q/K projection + RoPE for dim-tile `pair`.
                    Yields after each 8-matmul PSUM group (8 yields total)."""
                    for wts, outts, nm in ((wqts, qts, "q"), (wkts, kts, "k")):
                        for half in range(2):
                            tmp2 = rtp.tile([128, 1024], F32, tag="tmp2",
                                            name=f"tmp2{nm}{pair}{half}")
                            tmpc = rtp.tile([128, 1024], F32, tag="tmpc",
                                            name=f"tmpc{nm}{pair}{half}")
                            tmp3 = rtp.tile([128, 1024], F32, tag="tmp3",
                                            name=f"tmp3{nm}{pair}{half}")
                            for j in range(2):
                                tch = half * 2 + j
                                ps = prp.tile([128, 512], F32, tag="ps",
                                              name=f"ps{nm}{pair}{tch}")
                                for ht in range(HT):
                                    nc.tensor.matmul(
                                        ps[:],
                                        lhsT=wts[ht][:, pair * 128:(pair + 1) * 128],
                                        rhs=xts[ht][:, tch * 512:(tch + 1) * 512],
                                        start=(ht == 0), stop=(ht == HT - 1))
                                cs = slice(tch * 512, (tch + 1) * 512)
                                js = slice(j * 512, (j + 1) * 512)
                                nc.vector.tensor_mul(tmp2[:, js], ps[:], sinr_t[:, cs])
                                nc.vector.tensor_mul(tmpc[:, js], ps[:], cos_t[:, cs])
                                yield
                            hs_ = slice(half * 1024, (half + 1) * 1024)
                            for blk in (0, 64):
                                nc.sync.dma_start(tmp3[blk:blk + 32, :],
                                                  tmp2[blk + 32:blk + 64, :])
                                nc.sync.dma_start(tmp3[blk + 32:blk + 64, :],
                                                  tmp2[blk:blk + 32, :])
                            nc.vector.tensor_add(outts[pair][:, hs_], tmpc[:],
                                                 tmp3[:])

                LAG = 3  # PV trails exp by LAG k-tiles so PE never waits on ACT

                slots = [(h, qc) for h in range(NH_LOC) for qc in range(2)]
                n_units = len(slots) * TT
                pvs, pts = {}, {}

                def emit_qk_exp(u):
                    si, kt_i = divmod(u, TT)
                    h, qc = slots[si]
                    p, rb = h // 2, (h % 2) * 64
                    q0 = qc * 1024
                    qk = qkps.tile([128, 1024], F32, tag="qk", name=f"qk{u}")
                    for j in range(2):
                        nc.tensor.matmul(
                            qk[:, j * 512:(j + 1) * 512],
                            lhsT=kts[p][rb:rb + 64, kt_i * 128:(kt_i + 1) * 128],
                            rhs=qts[p][rb:rb + 64,
                                       q0 + j * 512:q0 + (j + 1) * 512],
                            start=True, stop=True)
                    pt = ptp.tile([128, 1024], BF16, tag="pt", name=f"pt{u}")
                    nc.scalar.activation(pt[:], qk[:], EXP, scale=SCALE)
                    pts[u] = pt

                def emit_pv(u):
                    si, kt_i = divmod(u, TT)
                    h, qc = slots[si]
                    if kt_i == 0:
                        pvs[si] = pvps.tile([VW, 1024], F32, tag="pv",
                                            name=f"pv{si}")
                    pv = pvs[si]
                    for j in range(2):
                        nc.tensor.matmul(
                            pv[:, j * 512:(j + 1) * 512],
                            lhsT=vps[kt_i][:, h * VW:(h + 1) * VW],
                            rhs=pts[u][:, j * 512:(j + 1) * 512],
                            start=(kt_i == 0), stop=(kt_i == TT - 1))
                    del pts[u]
                    if kt_i == TT - 1:
                        emit_epilogue(si)

                def emit_epilogue(si):
                    h, qc = slots[si]
                    p, rb = h // 2, (h % 2) * 64
                    q0 = qc * 1024
                    pv = pvs.pop(si)
                    # evacuate unnormalized A^T + sums; frees the PV bank fast,
                    # normalization stays off the PE critical path
                    au = aup.tile([HD, 1024], F32, tag="au", name=f"au{si}")
                    nc.vector.tensor_copy(au[:], pv[0:HD, :])
                    sums = rcp.tile([1, 1024], F32, tag="sums", name=f"sm{si}",
                                    bufs=1)
                    nc.vector.tensor_copy(sums[:], pv[HD:VW, :])
                    recip = rcp.tile([1, 1024], F32, tag="recip", name=f"rp{si}",
                                     bufs=1)
                    nc.vector.reciprocal_approx_fast(recip[:], sums[:])
                    recip64 = rcp.tile([64, 1024], F32, tag="recip64",
                                       name=f"rp64{si}", bufs=1)
                    nc.gpsimd.partition_broadcast(recip64[:], recip[:],
                                                  channels=64)
                    nc.vector.tensor_mul(ats[p][rb:rb + 64, q0:q0 + 1024],
                                         au[:], recip64[:])

                # ---- emission schedule -------------------------------------
                v_gen = emit_v_proj()
                for _ in range(LAG + 1):
                    next(v_gen, None)
                for _ in emit_qk_proj(0):
                    pass
                proj_gen = None
                for u in range(n_units + LAG):
                    if u < n_units:
                        si, kt_i = divmod(u, TT)
                        if kt_i == 0 and si % 4 == 0:
                            # entering a new dim-tile pair: its proj must be
                            # fully emitted; then start the next pair's proj
                            if proj_gen is not None:
                                for _ in proj_gen:
                                    pass
                            pair = si // 4 + 1
                            proj_gen = (iter(emit_qk_proj(pair))
                                        if pair < DT else None)
                        emit_qk_exp(u)
                        if si == 0:
                            next(v_gen, None)  # V-proj filler in first slot
                        elif proj_gen is not None and kt_i in (5, 13):
                            next(proj_gen, None)  # Q/K proj filler
                    if u >= LAG:
                        emit_pv(u - LAG)

            # ---- output projection ------------------------------------------
            with tc.tile_pool(name="o_ps", bufs=4, space="PSUM") as ops, \
                 tc.tile_pool(name="y", bufs=4) as yp:
                for ot in range(HT):
                    for tch in range(TCH):
                        ps = ops.tile([128, 512], F32, tag="ps", name=f"yps{ot}{tch}")
                        for dt_i in range(DT):
                            nc.tensor.matmul(
                                ps[:],
                                lhsT=wots[dt_i][:, ot * 128:(ot + 1) * 128],
                                rhs=ats[dt_i][:, tch * 512:(tch + 1) * 512],
                                start=(dt_i == 0), stop=(dt_i == DT - 1))
                        ysb = yp.tile([128, 512], F32, tag="y", name=f"y{ot}{tch}")
                        nc.vector.tensor_copy(ysb[:], ps[:])
                        nc.sync.dma_start(
                            yt[ot * 128:(ot + 1) * 128,
                               tch * 512:(tch + 1) * 512], ysb[:])

    nc.compile()
    return nc


_NC = None


def _get_nc():
    global _NC
    if _NC is None:
        _NC = _build_nc()
    return _NC


def _host_inputs(hidden_states, Wq, Wk, Wv, Wo):
    bf = ml_dtypes.bfloat16
    inv = 1.0 / (ROPE_BASE ** (np.arange(0, HD, 2, dtype=np.float64) / HD))
    t = np.arange(S, dtype=np.float64)
    ang = np.outer(inv, t)                      # [32, S]
    cos32 = np.cos(ang).astype(np.float32)
    sin32 = np.sin(ang).astype(np.float32)
    cosd = np.tile(cos32, (4, 1))               # [128, S]
    # sinrot: +sin on lower half of each 64-block, -sin on upper half
    sinrd = np.tile(np.concatenate([sin32, -sin32], axis=0), (2, 1))
    cosd = np.ascontiguousarray(cosd, dtype=np.float32)
    sinrd = np.ascontiguousarray(sinrd, dtype=np.float32)

    WqT = np.ascontiguousarray(Wq.T).astype(bf)     # [H, H]
    WkT = np.ascontiguousarray(Wk.T).astype(bf)
    WvT = np.ascontiguousarray(Wv.T).astype(bf)
    WoT = np.ascontiguousarray(Wo.T).astype(bf)     # [H(d), H(o)]

    in_maps = []
    for c in range(N_CORES):
        b, g = c // 2, c % 2
        gs = slice(g * DLOC, (g + 1) * DLOC)
        in_maps.append({
            "xt": np.ascontiguousarray(hidden_states[b].T).astype(bf),
            "wqt": np.ascontiguousarray(WqT[:, gs]),
            "wkt": np.ascontiguousarray(WkT[:, gs]),
            "wvt": np.ascontiguousarray(WvT[:, gs]),
            "wot": np.ascontiguousarray(WoT[gs, :]),
            "cosd": cosd,
            "sinrd": sinrd,
        })
    return in_maps


def kernel(hidden_states, Wq, Wk, Wv, Wo, _trace=False, _tmpdir=None):
    nc = _get_nc()
    in_maps = _host_inputs(hidden_states, Wq, Wk, Wv, Wo)
    res = run_bass_kernel_spmd(nc, in_maps, core_ids=list(range(N_CORES)),
                               trace=_trace, tmpdir=_tmpdir)
    kernel._last_results = res
    out = np.empty((B, S, H), dtype=np.float32)
    for b in range(B):
        acc = res.results[2 * b]["yt"].astype(np.float32) \
            + res.results[2 * b + 1]["yt"].astype(np.float32)
        out[b] = acc.T
    return out
